# revision 24
# baseline (speedup 1.0000x reference)
"""Trainium2 Bass kernel for the MoE-routed adapter problem.

Reference computation (M=2 routers, N=8 adapters, C=1024, D=256, B=32, S=512):
    per (m, b):  e = expert_index[m, b]
                 z = silu(x[b] @ down_w[m, e] + down_b[m, e])   # [S, D]
                 u = z @ up_w[m, e]                              # [S, C]
    out[m, b] = u                                                # [M, B, S, C]

Strategy: data-parallel over B across the 8 NeuronCores (4 batch elements per
core).  The expert gather is done host-side (numpy take_along_axis), weights
and activations are packed host-side into SBUF-layout contiguous blocks so
every device DMA is a single contiguous transfer.  Compute is bf16 matmuls
with fp32 PSUM accumulation (1 cycle/row on TensorE vs 4 for fp32); silu+bias
is fused into the ScalarEngine activation on the PSUM->SBUF path.

All matmuls are out[M,N] = lhsT[K,M].T @ rhs[K,N]:
  down: lhsT = down_w chunk [c:128, d:128], rhs = xT chunk [c:128, s:512]
        accumulated over 8 c-chunks -> zT [d:128, s:512] (transposed layout,
        exactly what the up matmul needs as its stationary operand)
  up:   lhsT = zT chunk [d:128, s:128], rhs = up_w chunk [d:128, c:512]
        accumulated over 2 d-chunks -> u [s:128, c:512] (natural layout)

Weight dedup: two local batches that route to the same expert of the same
router can share one weight tile.  SPMD needs an identical instruction stream
on every core, so a host-side solver re-groups the 32 batches into 8 groups
of 4 that all match one uniform duplicate pattern ("shape"):
  u6:  group = [x1, x2, y1, y2] with idx0[x1]==idx0[x2], idx1[y1]==idx1[y2]
       -> 6 weight slots/core instead of 8 (saves 2 MB of DMA per core)
  u7a/u7b: a single shared pair -> 7 slots;   u8: no sharing (fallback)
The compiled graph depends only on the shape; per-call content rides in the
in_maps, so recompiles only happen if a different input needs another shape.
"""

import sys

if "/opt/trn_rl_repo" not in sys.path:
    sys.path.insert(0, "/opt/trn_rl_repo")

import random

import numpy as np
import ml_dtypes

BF16 = ml_dtypes.bfloat16

M, N, C, D = 2, 8, 1024, 256
B, S = 32, 512
NCORES = 8
BL = B // NCORES  # local batch per core
CK = C // 128     # 8 contraction chunks for the down matmul
DK = D // 128     # 2 d chunks
SK = S // 128     # 4 s chunks
NC2 = C // 512    # 2 output column chunks of 512

# (m, bl) -> weight slot id, uniform across cores
SHAPES = {
    "u6": {(0, 0): 0, (0, 1): 0, (1, 0): 1, (1, 1): 2,
           (0, 2): 3, (0, 3): 4, (1, 2): 5, (1, 3): 5},
    "u7a": {(0, 0): 0, (0, 1): 0, (1, 0): 1, (1, 1): 2,
            (0, 2): 3, (1, 2): 4, (0, 3): 5, (1, 3): 6},
    "u7b": {(1, 0): 0, (1, 1): 0, (0, 0): 1, (0, 1): 2,
            (0, 2): 3, (1, 2): 4, (0, 3): 5, (1, 3): 6},
    "u8": {(m, bl): m * BL + bl for m in range(M) for bl in range(BL)},
}

_compiled = {}


# ----------------------------------------------------------------- planning

def _solve_u6(idx, trials=30000):
    """Split all 32 batches into 8 idx0-equal pairs + 8 idx1-equal pairs."""
    rng = random.Random(0)
    Bn = idx.shape[1]
    for _ in range(trials):
        order = list(range(Bn))
        rng.shuffle(order)
        p0_pend, p1_pend, pairs0, pairs1 = {}, {}, [], []
        ok = True
        for b in order:
            v0, v1 = int(idx[0, b]), int(idx[1, b])
            if v0 in p0_pend and len(pairs0) < Bn // 4:
                pairs0.append((p0_pend.pop(v0), b))
            elif v1 in p1_pend and len(pairs1) < Bn // 4:
                pairs1.append((p1_pend.pop(v1), b))
            elif len(pairs0) + len(p0_pend) < Bn // 4 and (
                rng.random() < 0.5 or len(pairs1) + len(p1_pend) >= Bn // 4
            ):
                if v0 in p0_pend:
                    ok = False
                    break
                p0_pend[v0] = b
            elif len(pairs1) + len(p1_pend) < Bn // 4:
                if v1 in p1_pend:
                    ok = False
                    break
                p1_pend[v1] = b
            else:
                ok = False
                break
        if ok and len(pairs0) == Bn // 4 and len(pairs1) == Bn // 4:
            groups = [
                [pairs0[c][0], pairs0[c][1], pairs1[c][0], pairs1[c][1]]
                for c in range(NCORES)
            ]
            return groups
    return None


def _solve_u7(idx, m):
    """8 disjoint idx[m]-equal pairs at bl 0/1; remaining batches fill bl 2/3."""
    byval = {}
    for b in range(idx.shape[1]):
        byval.setdefault(int(idx[m, b]), []).append(b)
    pairs = []
    rest = []
    for v, bs in byval.items():
        while len(bs) >= 2 and len(pairs) < NCORES:
            pairs.append((bs.pop(), bs.pop()))
        rest.extend(bs)
        byval[v] = []
    if len(pairs) < NCORES:
        return None
    for p in pairs[NCORES:]:
        rest.extend(p)
    pairs = pairs[:NCORES]
    groups = [
        [pairs[c][0], pairs[c][1], rest[2 * c], rest[2 * c + 1]]
        for c in range(NCORES)
    ]
    return groups


def _plan(idx):
    g = _solve_u6(idx)
    if g is not None:
        return "u6", g
    g = _solve_u7(idx, 0)
    if g is not None:
        return "u7a", g
    g = _solve_u7(idx, 1)
    if g is not None:
        return "u7b", g
    return "u8", [list(range(c * BL, (c + 1) * BL)) for c in range(NCORES)]


# ------------------------------------------------------------------ builder

def _build(shape_key):
    from concourse import bacc, tile, mybir

    f32 = mybir.dt.float32
    bf16 = mybir.dt.bfloat16
    Silu = mybir.ActivationFunctionType.Silu

    slotmap = SHAPES[shape_key]
    nslot = max(slotmap.values()) + 1
    first_use = {}
    for (m, bl), s in sorted(slotmap.items(), key=lambda kv: (kv[0][1], kv[0][0])):
        first_use.setdefault(s, (bl, m))

    nc = bacc.Bacc(
        "TRN2", target_bir_lowering=False, debug=False, num_devices=NCORES
    )

    # Per-core DRAM parameters, packed host-side into SBUF layout:
    #   xt : [BL, 128, CK*S]       xt[b][p, ck*S + s]              = x[b, s, ck*128+p]
    #   dw : [nslot, 128, 2048]    dw[s][p, dk*1024 + ck*128 + j]  = dw[ck*128+p, dk*128+j]
    #   uw : [nslot, 128, 2048]    uw[s][p, dk*1024 + c]           = uw[dk*128+p, c]
    #   db : [128, M*BL*DK]        db[p, (m*BL+b)*DK + dk]         = b_down[dk*128+p]
    #   out: [M, BL, 128, SK*C]    out[m,b][p, sk*C + c]           = u[sk*128+p, c]
    xt_d = nc.dram_tensor("xt", [BL, 128, CK * S], bf16, kind="ExternalInput")
    dw_d = nc.dram_tensor("dw", [nslot, 128, 2048], bf16, kind="ExternalInput")
    uw_d = nc.dram_tensor("uw", [nslot, 128, 2048], bf16, kind="ExternalInput")
    db_d = nc.dram_tensor("db", [128, M * BL * DK], f32, kind="ExternalInput")
    out_d = nc.dram_tensor("out", [M, BL, 128, SK * C], bf16, kind="ExternalOutput")

    with tile.TileContext(nc) as tc:
        with (
            tc.tile_pool(name="const", bufs=1) as cpool,
            tc.tile_pool(name="warm", bufs=1) as warmpool,
            tc.tile_pool(name="xp", bufs=4) as xpool,
            tc.tile_pool(name="wp", bufs=1) as wpool,
            tc.tile_pool(name="zp", bufs=3) as zpool,
            tc.tile_pool(name="up", bufs=3) as upool,
            tc.tile_pool(name="zpsum", bufs=2, space="PSUM") as zpsum,
            tc.tile_pool(name="upsum", bufs=6, space="PSUM") as upsum,
        ):
            # PE warm-up: dummy matmuls on memset scratch while the first
            # input DMAs are in flight, so HAM un-throttles the PE clock
            # before the real matmuls start (and the PE never sits cold).
            wsrc = warmpool.tile([128, 512], bf16, name="wsrc")
            nc.gpsimd.memset(wsrc[:], 0.0)
            pwarm = zpsum.tile([128, S], f32, name="pz")
            for _ in range(14):
                nc.tensor.matmul(
                    pwarm[:], wsrc[:, :128], wsrc[:], start=True, stop=True
                )

            dbt = cpool.tile([128, M * BL * DK], f32)

            slot_dw = {}
            slot_uw = {}

            def load_slot_dw(s, b):
                if s in slot_dw:
                    return
                if b == 0 and s == slotmap[(0, 0)]:
                    # b==0 m0 down weights ride the otherwise-idle Scalar
                    # HWDGE ring in dk halves so the first accumulation
                    # group unblocks as early as possible
                    da = wpool.tile([128, 1024], bf16, name=f"dwa{s}")
                    nc.scalar.dma_start(out=da[:], in_=dw_d.ap()[s][:, :1024])
                    db2 = wpool.tile([128, 1024], bf16, name=f"dwb{s}")
                    nc.scalar.dma_start(out=db2[:], in_=dw_d.ap()[s][:, 1024:])
                    slot_dw[s] = ("split", da, db2)
                else:
                    t = wpool.tile([128, 2048], bf16, name=f"dws{s}")
                    nc.gpsimd.dma_start(out=t[:], in_=dw_d.ap()[s])
                    slot_dw[s] = ("whole", t)

            def load_slot_uw(s):
                if s in slot_uw:
                    return
                u = wpool.tile([128, 2048], bf16, name=f"uws{s}")
                nc.gpsimd.dma_start(out=u[:], in_=uw_d.ap()[s])
                slot_uw[s] = u

            def dwsl(m, bl, dk, ck):
                rec = slot_dw[slotmap[(m, bl)]]
                if rec[0] == "split":
                    return rec[1 + dk][:, ck * 128 : (ck + 1) * 128]
                return rec[1][:, dk * 1024 + ck * 128 : dk * 1024 + (ck + 1) * 128]

            def uwsl(m, bl, dk, ncol):
                t = slot_uw[slotmap[(m, bl)]]
                return t[:, dk * 1024 + ncol * 512 : dk * 1024 + (ncol + 1) * 512]

            for b in range(BL):
                xts = []
                for h in range(2):
                    xh = xpool.tile([128, 4 * S], bf16, name="xt")
                    nc.sync.dma_start(
                        out=xh[:], in_=xt_d.ap()[b][:, h * 2048 : (h + 1) * 2048]
                    )
                    xts.append(xh)

                # new weight slots in PE consumption order: all down weights
                # (m0 before m1) first, then the bias table, then up weights
                new_slots = sorted(
                    (s for s, (bl, _m) in first_use.items() if bl == b),
                    key=lambda s: first_use[s][1],
                )
                for s in new_slots:
                    load_slot_dw(s, b)
                if b == 0:
                    nc.gpsimd.dma_start(out=dbt[:], in_=db_d.ap())
                for s in new_slots:
                    load_slot_uw(s)

                # down projection + silu for both routers first so the up
                # matmuls of router m overlap the activation of router m+1
                zts = []
                for m in range(M):
                    zt = zpool.tile([128, DK, S], bf16, name="zt")
                    if b == 0 and m == 0:
                        # x halves arrive ~3us apart on the Sync ring during
                        # the fill; interleave the two dk accumulation groups
                        # so the PE has 8 useful matmuls per arrived half
                        # instead of stalling mid-group
                        pzs = [zpsum.tile([128, S], f32, name="pz") for _ in range(DK)]
                        for blk in range(2):
                            for dk in range(DK):
                                for ck in range(blk * 4, blk * 4 + 4):
                                    nc.tensor.matmul(
                                        pzs[dk][:],
                                        dwsl(m, b, dk, ck),
                                        xts[blk][:, (ck % 4) * S : (ck % 4 + 1) * S],
                                        start=(ck == 0),
                                        stop=(ck == CK - 1),
                                    )
                        for dk in range(DK):
                            col = (m * BL + b) * DK + dk
                            nc.scalar.activation(
                                zt[:, dk, :], pzs[dk][:], Silu,
                                bias=dbt[:, col : col + 1],
                            )
                    else:
                        for dk in range(DK):
                            pz = zpsum.tile([128, S], f32, name="pz")
                            for ck in range(CK):
                                nc.tensor.matmul(
                                    pz[:],
                                    dwsl(m, b, dk, ck),
                                    xts[ck // 4][:, (ck % 4) * S : (ck % 4 + 1) * S],
                                    start=(ck == 0),
                                    stop=(ck == CK - 1),
                                )
                            col = (m * BL + b) * DK + dk
                            nc.scalar.activation(
                                zt[:, dk, :], pz[:], Silu, bias=dbt[:, col : col + 1]
                            )
                    zts.append(zt)

                for m in range(M):
                    ut = upool.tile([128, SK * C], bf16, name="ut")
                    for sk in range(SK):
                        pus = [
                            upsum.tile([128, 512], f32, name="pu") for _ in range(NC2)
                        ]
                        for dk in range(DK):
                            for ncol in range(NC2):
                                nc.tensor.matmul(
                                    pus[ncol][:],
                                    zts[m][:, dk, sk * 128 : (sk + 1) * 128],
                                    uwsl(m, b, dk, ncol),
                                    start=(dk == 0),
                                    stop=(dk == DK - 1),
                                )
                        # drain PSUM->SBUF(bf16) split across both engines so
                        # neither becomes the up-phase bottleneck
                        for ncol in range(NC2):
                            dst = ut[
                                :, sk * C + ncol * 512 : sk * C + (ncol + 1) * 512
                            ]
                            if ncol == 0:
                                nc.vector.tensor_copy(dst, pus[ncol][:])
                            else:
                                nc.scalar.copy(dst, pus[ncol][:])
                        last_tile = b == BL - 1 and m == M - 1
                        if last_tile:
                            if sk == SK - 1:
                                nc.sync.dma_start(
                                    out=out_d.ap()[m, b][:, sk * C : sk * C + 512],
                                    in_=ut[:, sk * C : sk * C + 512],
                                )
                                nc.scalar.dma_start(
                                    out=out_d.ap()[m, b][
                                        :, sk * C + 512 : (sk + 1) * C
                                    ],
                                    in_=ut[:, sk * C + 512 : (sk + 1) * C],
                                )
                            else:
                                eng = nc.sync if sk % 2 == 0 else nc.scalar
                                eng.dma_start(
                                    out=out_d.ap()[m, b][:, sk * C : (sk + 1) * C],
                                    in_=ut[:, sk * C : (sk + 1) * C],
                                )
                        else:
                            if sk == 1:
                                nc.scalar.dma_start(
                                    out=out_d.ap()[m, b][:, : 2 * C],
                                    in_=ut[:, : 2 * C],
                                )
                            if sk == SK - 1:
                                nc.sync.dma_start(
                                    out=out_d.ap()[m, b][:, 2 * C :],
                                    in_=ut[:, 2 * C :],
                                )

    nc.compile()
    return nc


def _get_compiled(shape_key):
    if shape_key not in _compiled:
        _compiled[shape_key] = _build(shape_key)
    return _compiled[shape_key]


# ------------------------------------------------------------------ runner

def _pack_inputs(x, expert_index, down_w, down_b, up_w, shape_key, groups):
    idx = expert_index.astype(np.int64)
    dwg = np.take_along_axis(down_w, idx[:, :, None, None], axis=1)  # [M,B,C,D]
    dbg = np.take_along_axis(down_b, idx[:, :, None], axis=1)        # [M,B,D]
    uwg = np.take_along_axis(up_w, idx[:, :, None, None], axis=1)    # [M,B,D,C]

    # x -> [B, 128, CK*S]: xt[b, p, ck*S+s] = x[b, s, ck*128+p]
    xt = (
        x.transpose(0, 2, 1)
        .reshape(B, CK, 128, S)
        .transpose(0, 2, 1, 3)
        .reshape(B, 128, CK * S)
        .astype(BF16)
    )
    # down_w -> [M, B, 128, 2048]: [p, dk*1024 + ck*128 + j] = dw[ck*128+p, dk*128+j]
    dwp = (
        dwg.reshape(M, B, CK, 128, DK, 128)
        .transpose(0, 1, 3, 4, 2, 5)
        .reshape(M, B, 128, 2048)
        .astype(BF16)
    )
    # up_w -> [M, B, 128, 2048]: [p, dk*1024 + c] = uw[dk*128+p, c]
    uwp = (
        uwg.reshape(M, B, DK, 128, C)
        .transpose(0, 1, 3, 2, 4)
        .reshape(M, B, 128, 2048)
        .astype(BF16)
    )

    slotmap = SHAPES[shape_key]
    nslot = max(slotmap.values()) + 1
    slot_rep = {}
    for (m, bl), s in slotmap.items():
        slot_rep.setdefault(s, (m, bl))

    in_maps = []
    for c in range(NCORES):
        gb = groups[c]
        dwc = np.stack([dwp[slot_rep[s][0], gb[slot_rep[s][1]]] for s in range(nslot)])
        uwc = np.stack([uwp[slot_rep[s][0], gb[slot_rep[s][1]]] for s in range(nslot)])
        dbc = (
            dbg[:, gb]
            .reshape(M, BL, DK, 128)
            .transpose(3, 0, 1, 2)
            .reshape(128, M * BL * DK)
            .astype(np.float32)
        )
        in_maps.append(
            {
                "xt": np.ascontiguousarray(xt[gb]),
                "dw": np.ascontiguousarray(dwc),
                "uw": np.ascontiguousarray(uwc),
                "db": np.ascontiguousarray(dbc),
            }
        )
    return in_maps


def kernel(x, expert_index, down_w, down_b, up_w, _run_kwargs=None):
    expert_index = np.asarray(expert_index)
    shape_key, groups = _plan(expert_index)
    nc = _get_compiled(shape_key)
    in_maps = _pack_inputs(
        np.asarray(x, dtype=np.float32),
        expert_index,
        np.asarray(down_w, dtype=np.float32),
        np.asarray(down_b, dtype=np.float32),
        np.asarray(up_w, dtype=np.float32),
        shape_key,
        groups,
    )

    from concourse.bass_utils import run_bass_kernel_spmd

    res = run_bass_kernel_spmd(
        nc, in_maps, core_ids=list(range(NCORES)), **(_run_kwargs or {})
    )

    out = np.empty((M, B, S, C), dtype=np.float32)
    for c in range(NCORES):
        buf = (
            np.asarray(res.results[c]["out"])          # [M, BL, 128, SK*C] bf16
            .astype(np.float32)
            .reshape(M, BL, 128, SK, C)
            .transpose(0, 1, 3, 2, 4)
            .reshape(M, BL, S, C)
        )
        for bl, gb in enumerate(groups[c]):
            out[:, gb] = buf[:, bl]
    globals()["_last_results"] = res
    return out


# revision 26
# speedup vs baseline: 1.1536x; 1.1536x over previous
"""Trainium2 Bass kernel for the MoE-routed adapter problem.

Reference computation (M=2 routers, N=8 adapters, C=1024, D=256, B=32, S=512):
    per (m, b):  e = expert_index[m, b]
                 z = silu(x[b] @ down_w[m, e] + down_b[m, e])   # [S, D]
                 u = z @ up_w[m, e]                              # [S, C]
    out[m, b] = u                                                # [M, B, S, C]

Strategy: data-parallel over B across the 8 NeuronCores (4 batch elements per
core).  The expert gather is done host-side (numpy take_along_axis), weights
and activations are packed host-side into SBUF-layout contiguous blocks so
every device DMA is a single contiguous transfer.  Compute is bf16 matmuls
with fp32 PSUM accumulation (1 cycle/row on TensorE vs 4 for fp32); silu+bias
is fused into the ScalarEngine activation on the PSUM->SBUF path.

All matmuls are out[M,N] = lhsT[K,M].T @ rhs[K,N]:
  down: lhsT = down_w chunk [c:128, d:128], rhs = xT chunk [c:128, s:512]
        accumulated over 8 c-chunks -> zT [d:128, s:512] (transposed layout,
        exactly what the up matmul needs as its stationary operand)
  up:   lhsT = zT chunk [d:128, s:128], rhs = up_w chunk [d:128, c:512]
        accumulated over 2 d-chunks -> u [s:128, c:512] (natural layout)

Weight dedup: two local batches that route to the same expert of the same
router can share one weight tile.  SPMD needs an identical instruction stream
on every core, so a host-side solver re-groups the 32 batches into 8 groups
of 4 that all match one uniform duplicate pattern ("shape"):
  u6:  group = [x1, x2, y1, y2] with idx0[x1]==idx0[x2], idx1[y1]==idx1[y2]
       -> 6 weight slots/core instead of 8 (saves 2 MB of DMA per core)
  u7a/u7b: a single shared pair -> 7 slots;   u8: no sharing (fallback)
The compiled graph depends only on the shape; per-call content rides in the
in_maps, so recompiles only happen if a different input needs another shape.
"""

import sys

if "/opt/trn_rl_repo" not in sys.path:
    sys.path.insert(0, "/opt/trn_rl_repo")

import random

import numpy as np
import ml_dtypes

BF16 = ml_dtypes.bfloat16

M, N, C, D = 2, 8, 1024, 256
B, S = 32, 512
NCORES = 8
BL = B // NCORES  # local batch per core
CK = C // 128     # 8 contraction chunks for the down matmul
DK = D // 128     # 2 d chunks
SK = S // 128     # 4 s chunks
NC2 = C // 512    # 2 output column chunks of 512

# (m, bl) -> weight slot id, uniform across cores
SHAPES = {
    "u6": {(0, 0): 0, (0, 1): 0, (1, 0): 1, (1, 1): 2,
           (0, 2): 3, (0, 3): 4, (1, 2): 5, (1, 3): 5},
    "u7a": {(0, 0): 0, (0, 1): 0, (1, 0): 1, (1, 1): 2,
            (0, 2): 3, (1, 2): 4, (0, 3): 5, (1, 3): 6},
    "u7b": {(1, 0): 0, (1, 1): 0, (0, 0): 1, (0, 1): 2,
            (0, 2): 3, (1, 2): 4, (0, 3): 5, (1, 3): 6},
    "u8": {(m, bl): m * BL + bl for m in range(M) for bl in range(BL)},
}

_compiled = {}


# ----------------------------------------------------------------- planning

def _solve_u6(idx, trials=30000):
    """Split all 32 batches into 8 idx0-equal pairs + 8 idx1-equal pairs."""
    rng = random.Random(0)
    Bn = idx.shape[1]
    for _ in range(trials):
        order = list(range(Bn))
        rng.shuffle(order)
        p0_pend, p1_pend, pairs0, pairs1 = {}, {}, [], []
        ok = True
        for b in order:
            v0, v1 = int(idx[0, b]), int(idx[1, b])
            if v0 in p0_pend and len(pairs0) < Bn // 4:
                pairs0.append((p0_pend.pop(v0), b))
            elif v1 in p1_pend and len(pairs1) < Bn // 4:
                pairs1.append((p1_pend.pop(v1), b))
            elif len(pairs0) + len(p0_pend) < Bn // 4 and (
                rng.random() < 0.5 or len(pairs1) + len(p1_pend) >= Bn // 4
            ):
                if v0 in p0_pend:
                    ok = False
                    break
                p0_pend[v0] = b
            elif len(pairs1) + len(p1_pend) < Bn // 4:
                if v1 in p1_pend:
                    ok = False
                    break
                p1_pend[v1] = b
            else:
                ok = False
                break
        if ok and len(pairs0) == Bn // 4 and len(pairs1) == Bn // 4:
            groups = [
                [pairs0[c][0], pairs0[c][1], pairs1[c][0], pairs1[c][1]]
                for c in range(NCORES)
            ]
            return groups
    return None


def _solve_u7(idx, m):
    """8 disjoint idx[m]-equal pairs at bl 0/1; remaining batches fill bl 2/3."""
    byval = {}
    for b in range(idx.shape[1]):
        byval.setdefault(int(idx[m, b]), []).append(b)
    pairs = []
    rest = []
    for v, bs in byval.items():
        while len(bs) >= 2 and len(pairs) < NCORES:
            pairs.append((bs.pop(), bs.pop()))
        rest.extend(bs)
        byval[v] = []
    if len(pairs) < NCORES:
        return None
    for p in pairs[NCORES:]:
        rest.extend(p)
    pairs = pairs[:NCORES]
    groups = [
        [pairs[c][0], pairs[c][1], rest[2 * c], rest[2 * c + 1]]
        for c in range(NCORES)
    ]
    return groups


def _plan(idx):
    g = _solve_u6(idx)
    if g is not None:
        return "u6", g
    g = _solve_u7(idx, 0)
    if g is not None:
        return "u7a", g
    g = _solve_u7(idx, 1)
    if g is not None:
        return "u7b", g
    return "u8", [list(range(c * BL, (c + 1) * BL)) for c in range(NCORES)]


# ------------------------------------------------------------------ builder

def _build(shape_key):
    from concourse import bacc, tile, mybir

    f32 = mybir.dt.float32
    bf16 = mybir.dt.bfloat16
    Silu = mybir.ActivationFunctionType.Silu

    slotmap = SHAPES[shape_key]
    nslot = max(slotmap.values()) + 1
    first_use = {}
    for (m, bl), s in sorted(slotmap.items(), key=lambda kv: (kv[0][1], kv[0][0])):
        first_use.setdefault(s, (bl, m))

    nc = bacc.Bacc(
        "TRN2", target_bir_lowering=False, debug=False, num_devices=NCORES
    )

    # Per-core DRAM parameters, packed host-side into SBUF layout:
    #   xt : [BL, 128, CK*S]       xt[b][p, ck*S + s]              = x[b, s, ck*128+p]
    #   dw : [nslot, 128, 2048]    dw[s][p, dk*1024 + ck*128 + j]  = dw[ck*128+p, dk*128+j]
    #   uw : [nslot, 128, 2048]    uw[s][p, dk*1024 + c]           = uw[dk*128+p, c]
    #   db : [128, M*BL*DK]        db[p, (m*BL+b)*DK + dk]         = b_down[dk*128+p]
    #   out: [M, BL, 128, SK*C]    out[m,b][p, sk*C + c]           = u[sk*128+p, c]
    xt_d = nc.dram_tensor("xt", [BL, 128, CK * S], bf16, kind="ExternalInput")
    dw_d = nc.dram_tensor("dw", [nslot, 128, 2048], bf16, kind="ExternalInput")
    uw_d = nc.dram_tensor("uw", [nslot, 128, 2048], bf16, kind="ExternalInput")
    db_d = nc.dram_tensor("db", [128, M * BL * DK], f32, kind="ExternalInput")
    out_d = nc.dram_tensor("out", [M, BL, 128, SK * C], bf16, kind="ExternalOutput")

    with tile.TileContext(nc) as tc:
        with (
            tc.tile_pool(name="const", bufs=1) as cpool,
            tc.tile_pool(name="warm", bufs=1) as warmpool,
            tc.tile_pool(name="xp", bufs=4) as xpool,
            tc.tile_pool(name="wp", bufs=1) as wpool,
            tc.tile_pool(name="zp", bufs=3) as zpool,
            tc.tile_pool(name="up", bufs=3) as upool,
            tc.tile_pool(name="zpsum", bufs=2, space="PSUM") as zpsum,
            tc.tile_pool(name="upsum", bufs=6, space="PSUM") as upsum,
        ):
            # PE warm-up: dummy matmuls on memset scratch while the first
            # input DMAs are in flight, so HAM un-throttles the PE clock
            # before the real matmuls start (and the PE never sits cold).
            wsrc = warmpool.tile([128, 512], bf16, name="wsrc")
            nc.gpsimd.memset(wsrc[:], 0.0)
            pwarm = zpsum.tile([128, S], f32, name="pz")
            for _ in range(14):
                nc.tensor.matmul(
                    pwarm[:], wsrc[:, :128], wsrc[:], start=True, stop=True
                )

            dbt = cpool.tile([128, M * BL * DK], f32)

            slot_dw = {}
            slot_uw = {}

            def load_slot_dw(s, b):
                if s in slot_dw:
                    return
                if b == 0 and s == slotmap[(0, 0)]:
                    # b==0 m0 down weights ride the otherwise-idle Scalar
                    # HWDGE ring in dk halves so the first accumulation
                    # group unblocks as early as possible
                    da = wpool.tile([128, 1024], bf16, name=f"dwa{s}")
                    nc.scalar.dma_start(out=da[:], in_=dw_d.ap()[s][:, :1024])
                    db2 = wpool.tile([128, 1024], bf16, name=f"dwb{s}")
                    nc.scalar.dma_start(out=db2[:], in_=dw_d.ap()[s][:, 1024:])
                    slot_dw[s] = ("split", da, db2)
                else:
                    t = wpool.tile([128, 2048], bf16, name=f"dws{s}")
                    nc.gpsimd.dma_start(out=t[:], in_=dw_d.ap()[s])
                    slot_dw[s] = ("whole", t)

            def load_slot_uw(s):
                if s in slot_uw:
                    return
                u = wpool.tile([128, 2048], bf16, name=f"uws{s}")
                nc.gpsimd.dma_start(out=u[:], in_=uw_d.ap()[s])
                slot_uw[s] = u

            def dwsl(m, bl, dk, ck):
                rec = slot_dw[slotmap[(m, bl)]]
                if rec[0] == "split":
                    return rec[1 + dk][:, ck * 128 : (ck + 1) * 128]
                return rec[1][:, dk * 1024 + ck * 128 : dk * 1024 + (ck + 1) * 128]

            def uwsl(m, bl, dk, ncol):
                t = slot_uw[slotmap[(m, bl)]]
                return t[:, dk * 1024 + ncol * 512 : dk * 1024 + (ncol + 1) * 512]

            for b in range(BL):
                if b == 0:
                    # quarter-granular x on the Sync ring: the first down
                    # matmuls unblock after only ~256 KB instead of 512 KB
                    xqs = []
                    for q in range(4):
                        xq = xpool.tile([128, 2 * S], bf16, name="xt4")
                        nc.sync.dma_start(
                            out=xq[:], in_=xt_d.ap()[b][:, q * 1024 : (q + 1) * 1024]
                        )
                        xqs.append(xq)
                    xslice = lambda ck, _x=xqs: _x[ck // 2][
                        :, (ck % 2) * S : (ck % 2 + 1) * S
                    ]
                else:
                    xts = []
                    for h in range(2):
                        xh = xpool.tile([128, 4 * S], bf16, name="xt")
                        nc.sync.dma_start(
                            out=xh[:], in_=xt_d.ap()[b][:, h * 2048 : (h + 1) * 2048]
                        )
                        xts.append(xh)
                    xslice = lambda ck, _x=xts: _x[ck // 4][
                        :, (ck % 4) * S : (ck % 4 + 1) * S
                    ]

                # new weight slots in PE consumption order: all down weights
                # (m0 before m1) first, then the bias table, then up weights
                new_slots = sorted(
                    (s for s, (bl, _m) in first_use.items() if bl == b),
                    key=lambda s: first_use[s][1],
                )
                for s in new_slots:
                    load_slot_dw(s, b)
                if b == 0:
                    nc.gpsimd.dma_start(out=dbt[:], in_=db_d.ap())
                for s in new_slots:
                    load_slot_uw(s)

                # down projection + silu for both routers first so the up
                # matmuls of router m overlap the activation of router m+1
                zts = []
                for m in range(M):
                    zt = zpool.tile([128, DK, S], bf16, name="zt")
                    for dk in range(DK):
                        pz = zpsum.tile([128, S], f32, name="pz")
                        for ck in range(CK):
                            nc.tensor.matmul(
                                pz[:],
                                dwsl(m, b, dk, ck),
                                xslice(ck),
                                start=(ck == 0),
                                stop=(ck == CK - 1),
                            )
                        col = (m * BL + b) * DK + dk
                        nc.scalar.activation(
                            zt[:, dk, :], pz[:], Silu, bias=dbt[:, col : col + 1]
                        )
                    zts.append(zt)

                for m in range(M):
                    ut = upool.tile([128, SK * C], bf16, name="ut")
                    for sk in range(SK):
                        pus = [
                            upsum.tile([128, 512], f32, name="pu") for _ in range(NC2)
                        ]
                        for dk in range(DK):
                            for ncol in range(NC2):
                                nc.tensor.matmul(
                                    pus[ncol][:],
                                    zts[m][:, dk, sk * 128 : (sk + 1) * 128],
                                    uwsl(m, b, dk, ncol),
                                    start=(dk == 0),
                                    stop=(dk == DK - 1),
                                )
                        # drain PSUM->SBUF(bf16) split across both engines so
                        # neither becomes the up-phase bottleneck
                        for ncol in range(NC2):
                            dst = ut[
                                :, sk * C + ncol * 512 : sk * C + (ncol + 1) * 512
                            ]
                            if ncol == 0:
                                nc.vector.tensor_copy(dst, pus[ncol][:])
                            else:
                                nc.scalar.copy(dst, pus[ncol][:])
                        last_tile = b == BL - 1 and m == M - 1
                        if last_tile:
                            if sk == SK - 1:
                                nc.sync.dma_start(
                                    out=out_d.ap()[m, b][:, sk * C : sk * C + 512],
                                    in_=ut[:, sk * C : sk * C + 512],
                                )
                                nc.scalar.dma_start(
                                    out=out_d.ap()[m, b][
                                        :, sk * C + 512 : (sk + 1) * C
                                    ],
                                    in_=ut[:, sk * C + 512 : (sk + 1) * C],
                                )
                            else:
                                eng = nc.sync if sk % 2 == 0 else nc.scalar
                                eng.dma_start(
                                    out=out_d.ap()[m, b][:, sk * C : (sk + 1) * C],
                                    in_=ut[:, sk * C : (sk + 1) * C],
                                )
                        else:
                            if sk == 1:
                                nc.scalar.dma_start(
                                    out=out_d.ap()[m, b][:, : 2 * C],
                                    in_=ut[:, : 2 * C],
                                )
                            if sk == SK - 1:
                                nc.sync.dma_start(
                                    out=out_d.ap()[m, b][:, 2 * C :],
                                    in_=ut[:, 2 * C :],
                                )

    nc.compile()
    return nc


def _get_compiled(shape_key):
    if shape_key not in _compiled:
        _compiled[shape_key] = _build(shape_key)
    return _compiled[shape_key]


# ------------------------------------------------------------------ runner

def _pack_inputs(x, expert_index, down_w, down_b, up_w, shape_key, groups):
    idx = expert_index.astype(np.int64)
    dwg = np.take_along_axis(down_w, idx[:, :, None, None], axis=1)  # [M,B,C,D]
    dbg = np.take_along_axis(down_b, idx[:, :, None], axis=1)        # [M,B,D]
    uwg = np.take_along_axis(up_w, idx[:, :, None, None], axis=1)    # [M,B,D,C]

    # x -> [B, 128, CK*S]: xt[b, p, ck*S+s] = x[b, s, ck*128+p]
    xt = (
        x.transpose(0, 2, 1)
        .reshape(B, CK, 128, S)
        .transpose(0, 2, 1, 3)
        .reshape(B, 128, CK * S)
        .astype(BF16)
    )
    # down_w -> [M, B, 128, 2048]: [p, dk*1024 + ck*128 + j] = dw[ck*128+p, dk*128+j]
    dwp = (
        dwg.reshape(M, B, CK, 128, DK, 128)
        .transpose(0, 1, 3, 4, 2, 5)
        .reshape(M, B, 128, 2048)
        .astype(BF16)
    )
    # up_w -> [M, B, 128, 2048]: [p, dk*1024 + c] = uw[dk*128+p, c]
    uwp = (
        uwg.reshape(M, B, DK, 128, C)
        .transpose(0, 1, 3, 2, 4)
        .reshape(M, B, 128, 2048)
        .astype(BF16)
    )

    slotmap = SHAPES[shape_key]
    nslot = max(slotmap.values()) + 1
    slot_rep = {}
    for (m, bl), s in slotmap.items():
        slot_rep.setdefault(s, (m, bl))

    in_maps = []
    for c in range(NCORES):
        gb = groups[c]
        dwc = np.stack([dwp[slot_rep[s][0], gb[slot_rep[s][1]]] for s in range(nslot)])
        uwc = np.stack([uwp[slot_rep[s][0], gb[slot_rep[s][1]]] for s in range(nslot)])
        dbc = (
            dbg[:, gb]
            .reshape(M, BL, DK, 128)
            .transpose(3, 0, 1, 2)
            .reshape(128, M * BL * DK)
            .astype(np.float32)
        )
        in_maps.append(
            {
                "xt": np.ascontiguousarray(xt[gb]),
                "dw": np.ascontiguousarray(dwc),
                "uw": np.ascontiguousarray(uwc),
                "db": np.ascontiguousarray(dbc),
            }
        )
    return in_maps


def kernel(x, expert_index, down_w, down_b, up_w, _run_kwargs=None):
    expert_index = np.asarray(expert_index)
    shape_key, groups = _plan(expert_index)
    nc = _get_compiled(shape_key)
    in_maps = _pack_inputs(
        np.asarray(x, dtype=np.float32),
        expert_index,
        np.asarray(down_w, dtype=np.float32),
        np.asarray(down_b, dtype=np.float32),
        np.asarray(up_w, dtype=np.float32),
        shape_key,
        groups,
    )

    from concourse.bass_utils import run_bass_kernel_spmd

    res = run_bass_kernel_spmd(
        nc, in_maps, core_ids=list(range(NCORES)), **(_run_kwargs or {})
    )

    out = np.empty((M, B, S, C), dtype=np.float32)
    for c in range(NCORES):
        buf = (
            np.asarray(res.results[c]["out"])          # [M, BL, 128, SK*C] bf16
            .astype(np.float32)
            .reshape(M, BL, 128, SK, C)
            .transpose(0, 1, 3, 2, 4)
            .reshape(M, BL, S, C)
        )
        for bl, gb in enumerate(groups[c]):
            out[:, gb] = buf[:, bl]
    globals()["_last_results"] = res
    return out
